# revision 1
# baseline (speedup 1.0000x reference)
"""GAT (2-layer, 8 heads) Trainium2 Bass kernel, sharded across 8 NeuronCores.

Sharding: nodes are partitioned into 8 contiguous ranges (graph parallel).
Edges are routed (on host) to the core that owns their dst node so that
segment-softmax and scatter-add stay local.  Between layers an AllGather
replicates the per-core [h | al_src] rows; the per-edge source-feature
fetch is a dma_gather (int16 indices, so the 50k-row table is split into
two <32k-row halves and each dst-tile's edge blocks are grouped by half).

Math note: the reference's segment-max subtraction is skipped — logits here
are O(1) so exp() cannot overflow, and alpha = e/z is invariant to the shift.

Self-contained: hardcodes all shapes from the problem spec.
"""

import os
import sys

import numpy as np

for _p in ("/opt/trn_rl_repo",):
    if _p not in sys.path and os.path.isdir(_p):
        sys.path.insert(0, _p)

import concourse.bacc as bacc
import concourse.bass as bass
import concourse.tile as tile
from concourse import ap_utils, bass_utils, mybir
from concourse.masks import make_identity

# ---------------- problem constants (from spec) ----------------
N = 50000
D_IN = 256
HID = 32
HEADS = 8
D = HEADS * HID  # 256
NEG_SLOPE = 0.2
NCORES = 8

NSH = N // NCORES  # 6250 nodes per core
P = 128
NT = (NSH + P - 1) // P  # 49 dst tiles per core
NSH_PAD = NT * P  # 6272
NFULL = NCORES * NSH_PAD  # 50176 rows in the AllGathered table
HALF = NFULL // 2  # 25088 (< 2**15, int16-indexable)
ROW = D + HEADS  # 264 floats gathered per edge ([h | al_src])
ROWP = 320  # padded DRAM row stride (1280 B, 256-aligned)
ADW = 64  # al_dst table row stride (256 B)
WCOL = D + 2 * HEADS  # 272 dense output cols ([h | al_src | al_dst])
G = 32  # edge blocks (of 128 edges) per processing group

f32 = mybir.dt.float32
i16 = mybir.dt.int16


def _ant_dma_gather(gp, out_ap, in_ap, idxs_ap, num_idxs, elem_size, elem_step,
                    queue_num=0):
    """bass.dma_gather (non-transpose, HBM source) without the elem%256 assert.

    The row stride (elem_step elements) must still be a multiple of 256 B.
    out[p, j, :] = in[idxs[j*128 + p], :elem_size]
    """
    assert idxs_ap.dtype == mybir.dt.int16
    assert in_ap.dtype == out_ap.dtype
    assert idxs_ap.space == bass.MemorySpace.SBUF
    assert out_ap.space == bass.MemorySpace.SBUF
    assert in_ap.space == bass.MemorySpace.DRAM
    assert ap_utils.ap_is_contiguous(out_ap.ap[1:])
    assert ap_utils.ap_is_contiguous(idxs_ap.ap[1:])
    assert num_idxs % 128 == 0
    assert out_ap.ap[-1][1] == elem_size
    assert out_ap.ap[0][1] * out_ap.ap[1][1] == num_idxs
    assert in_ap.ap[0][0] == elem_step
    stride_bytes = elem_step * mybir.dt.size(in_ap.dtype)
    stride_bytes_256 = stride_bytes // 256
    assert stride_bytes_256 * 256 == stride_bytes and stride_bytes_256 < 256
    _in_ap = gp.lower_ap_dma(in_ap, for_custom_bir_dma=True)
    return gp.add_instruction(
        mybir.InstDMAGatherAnt(
            name=gp.bass.get_next_instruction_name(),
            ins=[*_in_ap, gp.lower_ap(idxs_ap),
                 gp.lower_val_access(gp.to_reg(num_idxs))],
            outs=[gp.lower_ap(out_ap)],
            transpose=False,
            num_idxs=num_idxs,
            elem_size=elem_size,
            stride_bytes_256=stride_bytes_256,
            gen_mode=0,
            single_packet=True,
            queue_num=queue_num,
            sbuf_tokens_per_rank=0,
            sbuf_free_dim_per_rank=0,
            sbuf_free_dim_pad_per_rank=0,
            sbuf_byte_offset=0,
        )
    )


def _wrap16(vals_pb):
    """[128, btot] logical (partition, block) values -> [128, btot*8] int16
    dma_gather index layout (16-wrapped, replicated on all 8 channel groups)."""
    btot = vals_pb.shape[1]
    flat = vals_pb.T.reshape(-1)  # logical position q = blk*128 + p
    cols = flat.reshape(btot * 8, 16).T.astype(np.int16)  # [16, btot*8]
    return np.tile(cols, (8, 1))


# ---------------- host-side routing ----------------
def _route_edges(edge_index: np.ndarray):
    """Route edges (plus self-loops) to the core owning their dst; order each
    dst-tile's edges by src-table half so gather calls are half-homogeneous.

    Returns per-core index arrays plus the shared block/run schedule.
    """
    src = np.concatenate([edge_index[0].astype(np.int64), np.arange(N, dtype=np.int64)])
    dst = np.concatenate([edge_index[1].astype(np.int64), np.arange(N, dtype=np.int64)])
    core = dst // NSH
    dloc = dst - core * NSH
    g_row = (src // NSH) * NSH_PAD + (src % NSH)  # row in AllGathered table
    half = (g_row >= HALF).astype(np.int64)

    # per (core, tile, half) edge lists
    buckets = {}
    counts = np.zeros((NCORES, NT, 2), dtype=np.int64)
    for c in range(NCORES):
        m = core == c
        gr_c, dl_c, hf_c = g_row[m], dloc[m], half[m]
        key = dl_c // P * 2 + hf_c
        order = np.argsort(key, kind="stable")
        gr_c, dl_c, hf_c = gr_c[order], dl_c[order], hf_c[order]
        key = key[order]
        bounds = np.searchsorted(key, np.arange(NT * 2 + 1))
        for t in range(NT):
            for hf in range(2):
                lo, hi = bounds[t * 2 + hf], bounds[t * 2 + hf + 1]
                buckets[(c, t, hf)] = (gr_c[lo:hi], dl_c[lo:hi])
                counts[c, t, hf] = hi - lo

    B = -(-counts.max(axis=0) // P)  # [NT, 2] blocks per (tile, half)
    # block schedule: per tile, halves ordered to merge runs across tiles
    sched = []  # (tile, half)
    for t in range(NT):
        order = (0, 1) if t % 2 == 0 else (1, 0)
        for hf in order:
            sched.extend([(t, hf)] * int(B[t, hf]))
    # pad to a multiple of G, extending the final (tile, half) span
    btot = len(sched)
    pad = (-btot) % G
    sched.extend([sched[-1]] * pad)
    btot = len(sched)

    tile_of_block = np.array([t for t, _ in sched])
    half_of_block = np.array([hf for _, hf in sched])
    first = np.zeros(btot, dtype=bool)
    last = np.zeros(btot, dtype=bool)
    for t in range(NT):
        w = np.where(tile_of_block == t)[0]
        first[w[0]] = True
        last[w[-1]] = True

    # gather runs: maximal same-half spans, split at group boundaries
    runs = []  # (start_block, n_blocks, half)
    b = 0
    while b < btot:
        e = b + 1
        while (e < btot and half_of_block[e] == half_of_block[b]
               and e % G != 0):
            e += 1
        runs.append((b, e - b, int(half_of_block[b])))
        b = e

    # per-core per-(p, block) values
    srcv = np.zeros((NCORES, 128, btot), dtype=np.int64)
    dstl = np.full((NCORES, 128, btot), -1.0, dtype=np.float32)
    adst = np.zeros((NCORES, 128, btot), dtype=np.int64)
    blk_start = {}
    pos = 0
    for i, (t, hf) in enumerate(sched):
        if (t, hf) not in blk_start:
            blk_start[(t, hf)] = i
    for c in range(NCORES):
        for t in range(NT):
            for hf in range(2):
                gr, dl = buckets[(c, t, hf)]
                n_e = len(gr)
                if n_e == 0:
                    continue
                b0 = blk_start[(t, hf)]
                j = b0 * 128 + np.arange(n_e)
                srcv[c, j % 128, j // 128] = gr - hf * HALF
                dstl[c, j % 128, j // 128] = (dl - t * P).astype(np.float32)
                adst[c, j % 128, j // 128] = dl
    return srcv, dstl, adst, tile_of_block, first, last, runs, btot


# ---------------- device program ----------------
def _build_program(tile_of_block, first, last, runs, btot, phases=5):
    import contextlib

    nc = bacc.Bacc(
        "TRN2",
        target_bir_lowering=False,
        debug=False,
        enable_asserts=False,
        num_devices=NCORES,
        num_swdge_queues=4,
    )
    ngroups = btot // G
    qrr = [0]  # round-robin SWDGE queue assignment for gather calls

    def next_q():
        qrr[0] = (qrr[0] + 1) % 4
        return qrr[0]

    # I/O
    xT = nc.dram_tensor("xT", [D_IN, NSH_PAD], f32, kind="ExternalInput")
    xTf = nc.dram_tensor("xTf", [D_IN, NFULL], f32, kind="ExternalInput")
    w1 = nc.dram_tensor("w1", [D_IN, WCOL], f32, kind="ExternalInput")
    w2 = nc.dram_tensor("w2", [D, WCOL], f32, kind="ExternalInput")
    b1d = nc.dram_tensor("b1", [P, D], f32, kind="ExternalInput")
    b2d = nc.dram_tensor("b2", [P, HID], f32, kind="ExternalInput")
    iotad = nc.dram_tensor("iota", [P, P], f32, kind="ExternalInput")
    srcd = nc.dram_tensor("srci", [128, btot * 8], i16, kind="ExternalInput")
    dstld = nc.dram_tensor("dstl", [128, btot], f32, kind="ExternalInput")
    adstd = nc.dram_tensor("adst", [128, btot * 8], i16, kind="ExternalInput")
    outd = nc.dram_tensor("out", [NSH_PAD, HID], f32, kind="ExternalOutput")

    # internal DRAM
    haug_sh = [
        nc.dram_tensor(f"haug{l}_sh", [NSH_PAD, ROWP], f32, kind="Internal")
        for l in (1, 2)
    ]
    aldst_sh = [
        nc.dram_tensor(f"aldst{l}_sh", [NSH_PAD, ADW], f32, kind="Internal")
        for l in (1, 2)
    ]
    haug_full = [
        nc.dram_tensor(
            f"haug{l}_full", [NFULL, ROWP], f32, kind="Internal",
            addr_space=os.environ.get("HFULL_SPACE", "Shared"),
        )
        for l in (1, 2)
    ]

    rg = [list(range(NCORES))]

    with tile.TileContext(nc) as tc, contextlib.ExitStack() as ctx:
        const = ctx.enter_context(tc.tile_pool(name="const", bufs=1))
        sb = ctx.enter_context(tc.tile_pool(name="sb", bufs=2))
        sb3 = ctx.enter_context(tc.tile_pool(name="sb3", bufs=3))
        ps = ctx.enter_context(tc.tile_pool(name="ps", bufs=2, space="PSUM"))

        # constants
        ident = const.tile([P, P], f32)
        make_identity(nc, ident[:])
        w1t = const.tile([P, 2, WCOL], f32)
        nc.sync.dma_start(w1t[:], w1.ap().rearrange("(k p) n -> p k n", p=P))
        w2t = const.tile([P, 2, WCOL], f32)
        nc.sync.dma_start(w2t[:], w2.ap().rearrange("(k p) n -> p k n", p=P))
        b1t = const.tile([P, D], f32)
        nc.sync.dma_start(b1t[:], b1d.ap())
        b2t = const.tile([P, HID], f32)
        nc.sync.dma_start(b2t[:], b2d.ap())
        iota = const.tile([P, P], f32)
        nc.sync.dma_start(iota[:], iotad.ap())
        srci_sb = const.tile([128, btot * 8], i16)
        nc.sync.dma_start(srci_sb[:], srcd.ap())
        dstl_sb = const.tile([128, btot], f32)
        nc.sync.dma_start(dstl_sb[:], dstld.ap())
        adst_sb = const.tile([128, btot * 8], i16)
        nc.sync.dma_start(adst_sb[:], adstd.ap())

        def dense_tile(nt, lhsT_k0, lhsT_k1, wt, layer):
            """[128 nodes] x Waug matmul -> write haug/aldst shard rows."""
            d_ps = ps.tile([P, WCOL], f32, tag="dmm", name="d_ps")
            nc.tensor.matmul(d_ps[:], lhsT=lhsT_k0, rhs=wt[:, 0, :], start=True,
                             stop=False)
            nc.tensor.matmul(d_ps[:], lhsT=lhsT_k1, rhs=wt[:, 1, :], start=False,
                             stop=True)
            hb = sb.tile([P, WCOL], f32, tag="hb", name="hb")
            nc.vector.tensor_copy(hb[:], d_ps[:])
            r0, r1 = nt * P, (nt + 1) * P
            nc.sync.dma_start(haug_sh[layer].ap()[r0:r1, 0:ROW], hb[:, 0:ROW])
            nc.sync.dma_start(aldst_sh[layer].ap()[r0:r1, 0:HEADS],
                              hb[:, ROW:WCOL])

        # ---- dense layer 1, replicated: full [h | al_src] table, no AllGather
        xTfr = xTf.ap().rearrange("(k p) n -> p k n", p=P)
        for nt in range(NT * NCORES):
            xt = sb.tile([P, 2, P], f32, tag="xt", name="xt")
            nc.sync.dma_start(xt[:], xTfr[:, :, nt * P:(nt + 1) * P])
            d_ps = ps.tile([P, ROW], f32, tag="dmm", name="d_ps1")
            nc.tensor.matmul(d_ps[:], lhsT=xt[:, 0, :], rhs=w1t[:, 0, 0:ROW],
                             start=True, stop=False)
            nc.tensor.matmul(d_ps[:], lhsT=xt[:, 1, :], rhs=w1t[:, 1, 0:ROW],
                             start=False, stop=True)
            hb = sb.tile([P, ROW], f32, tag="hbf", name="hbf")
            nc.vector.tensor_copy(hb[:], d_ps[:])
            nc.sync.dma_start(haug_full[0].ap()[nt * P:(nt + 1) * P, 0:ROW],
                              hb[:])
        # ---- dense layer 1, own shard: local al_dst table only
        xTr = xT.ap().rearrange("(k p) n -> p k n", p=P)
        for nt in range(NT):
            xt = sb.tile([P, 2, P], f32, tag="xt", name="xt")
            nc.sync.dma_start(xt[:], xTr[:, :, nt * P:(nt + 1) * P])
            a_ps = ps.tile([P, HEADS], f32, tag="dmm", name="a_ps")
            nc.tensor.matmul(a_ps[:], lhsT=xt[:, 0, :], rhs=w1t[:, 0, ROW:WCOL],
                             start=True, stop=False)
            nc.tensor.matmul(a_ps[:], lhsT=xt[:, 1, :], rhs=w1t[:, 1, ROW:WCOL],
                             start=False, stop=True)
            ab = sb.tile([P, HEADS], f32, tag="ab", name="ab")
            nc.vector.tensor_copy(ab[:], a_ps[:])
            nc.sync.dma_start(aldst_sh[0].ap()[nt * P:(nt + 1) * P, 0:HEADS],
                              ab[:])

        def all_gather(layer):
            nc.gpsimd.collective_compute(
                "AllGather",
                mybir.AluOpType.bypass,
                replica_groups=rg,
                ins=[haug_sh[layer].ap()],
                outs=[haug_full[layer].ap()],
            )

        # SWDGE descriptor ring holds 128 descs/engine; cap calls well below.
        CHUNK = int(os.environ.get("CHUNK", "8"))  # blocks per dma_gather call

        def edge_phase(layer, flush):
            glimit = int(os.environ.get("GLIMIT", str(ngroups)))
            which = os.environ.get("EDGE_GATHER", "both")
            acc = {}
            run_i = 0
            for q in range(min(ngroups, glimit)):
                qs = q * G
                hs = sb.tile([128, G, ROW], f32, tag="hs", name="hs")
                while run_i < len(runs) and runs[run_i][0] < qs + G:
                    b0, nb, hf = runs[run_i]
                    for c0 in range(0, nb, CHUNK):
                        cb0, cnb = b0 + c0, min(CHUNK, nb - c0)
                        if which in ("both", "big"):
                            _ant_dma_gather(
                                nc.gpsimd,
                                out_ap=hs[:, cb0 - qs:cb0 - qs + cnb, :],
                                in_ap=haug_full[layer].ap()[
                                    hf * HALF:(hf + 1) * HALF, :],
                                idxs_ap=srci_sb[:, cb0 * 8:(cb0 + cnb) * 8],
                                num_idxs=cnb * 128,
                                elem_size=ROW,
                                elem_step=ROWP,
                                queue_num=next_q(),
                            )
                    run_i += 1
                ad = sb.tile([128, G, 8], f32, tag="ad", name="ad")
                if which in ("both", "ad"):
                    for c0 in range(0, G, CHUNK):
                        cnb = min(CHUNK, G - c0)
                        _ant_dma_gather(
                            nc.gpsimd,
                            out_ap=ad[:, c0:c0 + cnb, :],
                            in_ap=aldst_sh[layer].ap(),
                            idxs_ap=adst_sb[:, (qs + c0) * 8:(qs + c0 + cnb) * 8],
                            num_idxs=cnb * 128,
                            elem_size=8,
                            elem_step=ADW,
                            queue_num=next_q(),
                        )
                if which != "both":
                    o1 = sb.tile([P, HID], f32, tag="o1", name="o1")
                    src_t = hs if which == "big" else ad
                    nc.vector.tensor_copy(o1[:], src_t[:, 0, 0:HID] if which == "big"
                                          else ad[:, :, 0:1].rearrange(
                                              "p g c -> p (g c)")[:, 0:HID])
                    nc.sync.dma_start(outd.ap()[(q % NT) * P:(q % NT + 1) * P, :],
                                      o1[:])
                    continue
                emode = int(os.environ.get("EDGE_MODE", "4"))
                if emode == 1:
                    o1 = sb.tile([P, HID], f32, tag="o1", name="o1")
                    nc.vector.tensor_copy(o1[:], hs[:, 0, 0:HID])
                    nc.vector.tensor_copy(o1[:], ad[:, :, 0:1].rearrange(
                        "p g c -> p (g c)")[:, 0:HID])
                    nc.sync.dma_start(outd.ap()[q * P:(q + 1) * P, :], o1[:])
                    continue
                # logits = leaky_relu(al_src[src] + al_dst[dst]); w = exp(logits)
                lg = sb.tile([128, G, HEADS], f32, tag="lg", name="lg")
                nc.vector.tensor_tensor(
                    out=lg[:], in0=hs[:, :, D:ROW], in1=ad[:],
                    op=mybir.AluOpType.add,
                )
                lr = sb.tile([128, G, HEADS], f32, tag="lr", name="lr")
                nc.vector.tensor_scalar_mul(lr[:], lg[:], NEG_SLOPE)
                nc.vector.tensor_tensor(
                    out=lr[:], in0=lg[:], in1=lr[:], op=mybir.AluOpType.max
                )
                w = sb.tile([128, G, HEADS], f32, tag="w", name="w")
                nc.scalar.activation(w[:], lr[:], mybir.ActivationFunctionType.Exp)
                # scale h rows by the per-head weight; stash w in the al_src slot
                nc.vector.tensor_tensor(
                    out=hs[:, :, 0:D].rearrange("p g (h c) -> p g h c", c=HID),
                    in0=hs[:, :, 0:D].rearrange("p g (h c) -> p g h c", c=HID),
                    in1=w[:].unsqueeze(3).to_broadcast([128, G, HEADS, HID]),
                    op=mybir.AluOpType.mult,
                )
                nc.vector.tensor_copy(hs[:, :, D:ROW], w[:])
                # selection matrix S[p, :, d] = (dstl[p, :] == d)
                s = sb.tile([128, G, P], f32, tag="s", name="s")
                nc.vector.tensor_tensor(
                    out=s[:],
                    in0=dstl_sb[:, qs:qs + G].unsqueeze(2).to_broadcast([128, G, P]),
                    in1=iota[:].unsqueeze(1).to_broadcast([128, G, P]),
                    op=mybir.AluOpType.is_equal,
                )
                if emode == 2:
                    o1 = sb.tile([P, HID], f32, tag="o1", name="o1")
                    nc.vector.tensor_copy(o1[:], s[:, 0, 0:HID])
                    nc.vector.tensor_copy(o1[:], hs[:, 0, 0:HID])
                    nc.sync.dma_start(outd.ap()[q * P:(q + 1) * P, :], o1[:])
                    continue
                for g in range(G):
                    b = qs + g
                    t = int(tile_of_block[b])
                    if first[b]:
                        acc[t] = ps.tile([P, ROW], f32, tag="acc", name=f"acc{t}")
                    nc.tensor.matmul(
                        acc[t][:], lhsT=s[:, g, :], rhs=hs[:, g, :],
                        start=bool(first[b]), stop=bool(last[b]),
                    )
                    if last[b]:
                        if emode == 3:
                            o1 = sb.tile([P, HID], f32, tag="o1", name="o1")
                            nc.vector.tensor_copy(o1[:], acc.pop(t)[:, 0:HID])
                            nc.sync.dma_start(outd.ap()[t * P:(t + 1) * P, :], o1[:])
                        else:
                            flush(t, acc.pop(t))

        # ---- layer-1 flush: normalize + relu, then fused dense layer 2 ----
        def flush1(t, acc_ps):
            rz = sb.tile([P, HEADS], f32, tag="rz", name="rz")
            nc.vector.reciprocal(rz[:], acc_ps[:, D:ROW])
            h1 = sb3.tile([P, D], f32, tag="h1", name="h1")
            nc.vector.tensor_tensor(
                out=h1[:].rearrange("p (h c) -> p h c", c=HID),
                in0=acc_ps[:, 0:D].rearrange("p (h c) -> p h c", c=HID),
                in1=rz[:].unsqueeze(2).to_broadcast([P, HEADS, HID]),
                op=mybir.AluOpType.mult,
            )
            nc.vector.tensor_tensor(
                out=h1[:], in0=h1[:], in1=b1t[:], op=mybir.AluOpType.add
            )
            nc.vector.tensor_scalar_max(h1[:], h1[:], 0.0)
            # transpose h1 -> lhsT tiles for the layer-2 dense matmul
            tp0 = ps.tile([P, P], f32, tag="tp", name="tp0")
            nc.tensor.transpose(tp0[:], h1[:, 0:P], ident[:])
            l0 = sb.tile([P, P], f32, tag="l0", name="l0")
            nc.vector.tensor_copy(l0[:], tp0[:])
            tp1 = ps.tile([P, P], f32, tag="tp", name="tp1")
            nc.tensor.transpose(tp1[:], h1[:, P:D], ident[:])
            l1 = sb.tile([P, P], f32, tag="l1", name="l1")
            nc.vector.tensor_copy(l1[:], tp1[:])
            dense_tile(t, l0[:], l1[:], w2t, 1)

        # ---- layer-2 flush: normalize, mean over heads, + b2, store ----
        def flush2(t, acc_ps):
            rz = sb.tile([P, HEADS], f32, tag="rz", name="rz")
            nc.vector.reciprocal(rz[:], acc_ps[:, D:ROW])
            nc.vector.tensor_scalar_mul(rz[:], rz[:], 1.0 / HEADS)
            t2 = sb.tile([P, D], f32, tag="t2", name="t2")
            nc.vector.tensor_tensor(
                out=t2[:].rearrange("p (h c) -> p h c", c=HID),
                in0=acc_ps[:, 0:D].rearrange("p (h c) -> p h c", c=HID),
                in1=rz[:].unsqueeze(2).to_broadcast([P, HEADS, HID]),
                op=mybir.AluOpType.mult,
            )
            o = sb.tile([P, HID], f32, tag="o", name="o")
            nc.vector.tensor_reduce(
                out=o[:],
                in_=t2[:].rearrange("p (h c) -> p c h", c=HID),
                axis=mybir.AxisListType.X,
                op=mybir.AluOpType.add,
            )
            nc.vector.tensor_tensor(
                out=o[:], in0=o[:], in1=b2t[:], op=mybir.AluOpType.add
            )
            nc.sync.dma_start(outd.ap()[t * P:(t + 1) * P, :], o[:])

        repeat = int(os.environ.get("REPEAT", "1"))
        for _r in range(repeat):
            if phases >= 3:
                edge_phase(0, flush1)
            if phases >= 4:
                all_gather(1)
            if phases >= 5:
                edge_phase(1, flush2)

    nc.compile()
    return nc


# ---------------- public entry point ----------------
_CACHE = {}


def _prepare(edge_index):
    key = edge_index.tobytes()[:1024], int(edge_index.sum())
    if _CACHE.get("key") == key:
        return _CACHE["val"]
    srcv, dstl, adst, tile_of_block, first, last, runs, btot = _route_edges(
        np.asarray(edge_index)
    )
    nc = _build_program(tile_of_block, first, last, runs, btot)
    _CACHE["key"] = key
    _CACHE["val"] = (srcv, dstl, adst, btot, nc)
    return _CACHE["val"]


def _waug(W, a_src, a_dst):
    W = np.asarray(W, np.float32)
    asrc_m = np.zeros((D, HEADS), np.float32)
    adst_m = np.zeros((D, HEADS), np.float32)
    for h in range(HEADS):
        asrc_m[h * HID:(h + 1) * HID, h] = np.asarray(a_src, np.float32)[h]
        adst_m[h * HID:(h + 1) * HID, h] = np.asarray(a_dst, np.float32)[h]
    return np.concatenate([W, W @ asrc_m, W @ adst_m], axis=1)


def _make_in_maps(inputs, srcv, dstl, adst):
    x = np.asarray(inputs["x"], np.float32)
    w1_np = _waug(inputs["W1"], inputs["a_src1"], inputs["a_dst1"])
    w2_np = _waug(inputs["W2"], inputs["a_src2"], inputs["a_dst2"])
    b1_np = np.tile(np.asarray(inputs["b1"], np.float32).reshape(1, D), (P, 1))
    b2_np = np.tile(np.asarray(inputs["b2"], np.float32).reshape(1, HID), (P, 1))
    iota_np = np.tile(np.arange(P, dtype=np.float32).reshape(1, P), (P, 1))

    xT = np.zeros((NCORES, D_IN, NSH_PAD), np.float32)
    xt_full = np.ascontiguousarray(x.T)  # [256, 50000]
    for c in range(NCORES):
        xT[c, :, :NSH] = xt_full[:, c * NSH:(c + 1) * NSH]
    xTfull = np.zeros((D_IN, NFULL), np.float32)
    for c in range(NCORES):
        xTfull[:, c * NSH_PAD:c * NSH_PAD + NSH] = xt_full[:, c * NSH:(c + 1) * NSH]

    return [
        {
            "xT": np.ascontiguousarray(xT[c]),
            "xTf": xTfull,
            "w1": w1_np,
            "w2": w2_np,
            "b1": b1_np,
            "b2": b2_np,
            "iota": iota_np,
            "srci": _wrap16(srcv[c]),
            "dstl": np.ascontiguousarray(dstl[c]),
            "adst": _wrap16(adst[c]),
        }
        for c in range(NCORES)
    ]


def kernel(
    x, edge_index, W1, a_src1, a_dst1, b1, W2, a_src2, a_dst2, b2
) -> np.ndarray:
    inputs = dict(x=x, W1=W1, a_src1=a_src1, a_dst1=a_dst1, b1=b1,
                  W2=W2, a_src2=a_src2, a_dst2=a_dst2, b2=b2)
    srcv, dstl, adst, btot, nc = _prepare(np.asarray(edge_index))
    in_maps = _make_in_maps(inputs, srcv, dstl, adst)
    res = bass_utils.run_bass_kernel_spmd(nc, in_maps, core_ids=list(range(NCORES)))
    out = np.concatenate(
        [np.asarray(res.results[c]["out"])[:NSH] for c in range(NCORES)], axis=0
    )
    return out



# revision 8
# speedup vs baseline: 1.5153x; 1.5153x over previous
"""GAT (2-layer, 8 heads) Trainium2 Bass kernel, sharded across 8 NeuronCores.

Sharding: nodes are partitioned into 8 contiguous ranges (graph parallel).
Edges are routed (on host) to the core that owns their dst node so that
segment-softmax and scatter-add stay local.  Each layer's dense part runs on
the owning core only; an AllGather then replicates the per-core
[h | al_src] rows (bf16) so the per-edge source-feature fetch is a local
dma_gather (int16 indices; the 50k-row table is split into two <32k-row
halves and each dst-tile's edge blocks are grouped by half).

Math note: the reference's segment-max subtraction is skipped — logits here
are O(1) so exp() cannot overflow, and alpha = e/z is invariant to the shift.

Self-contained: hardcodes all shapes from the problem spec.
"""

import os
import sys

import numpy as np

for _p in ("/opt/trn_rl_repo",):
    if _p not in sys.path and os.path.isdir(_p):
        sys.path.insert(0, _p)

import ml_dtypes

import concourse.bacc as bacc
import concourse.bass as bass
import concourse.tile as tile
from concourse import ap_utils, bass_utils, mybir
from concourse.masks import make_identity

# ---------------- problem constants (from spec) ----------------
N = 50000
D_IN = 256
HID = 32
HEADS = 8
D = HEADS * HID  # 256
NEG_SLOPE = 0.2
NCORES = 8

NSH = N // NCORES  # 6250 nodes per core
P = 128
NT = (NSH + P - 1) // P  # 49 dst tiles per core
NSH_PAD = NT * P  # 6272
NFULL = NCORES * NSH_PAD  # 50176 rows in the AllGathered table
HALF = NFULL // 2  # 25088 (< 2**15, int16-indexable)
ROW = D + HEADS  # 264 bf16 gathered per edge ([h | al_src])
ROWP = 384  # padded DRAM row stride in bf16 elems (768 B, 256-aligned)
ADW = 128  # al_dst table row stride in bf16 elems (256 B)
WCOL = D + 2 * HEADS  # 272 dense output cols ([h | al_src | al_dst])
G = 32  # edge blocks (of 128 edges) per processing group

f32 = mybir.dt.float32
bf16 = mybir.dt.bfloat16
i16 = mybir.dt.int16
bfnp = ml_dtypes.bfloat16


def _ant_dma_gather(gp, out_ap, in_ap, idxs_ap, num_idxs, elem_size, elem_step,
                    queue_num=0):
    """bass.dma_gather (non-transpose, HBM source) without the elem%256 assert.

    The row stride (elem_step elements) must still be a multiple of 256 B.
    out[p, j, :] = in[idxs[j*128 + p], :elem_size]
    """
    assert idxs_ap.dtype == mybir.dt.int16
    assert in_ap.dtype == out_ap.dtype
    assert idxs_ap.space == bass.MemorySpace.SBUF
    assert out_ap.space == bass.MemorySpace.SBUF
    assert in_ap.space == bass.MemorySpace.DRAM
    assert ap_utils.ap_is_contiguous(out_ap.ap[1:])
    assert ap_utils.ap_is_contiguous(idxs_ap.ap[1:])
    assert num_idxs % 128 == 0
    assert out_ap.ap[-1][1] == elem_size
    assert out_ap.ap[0][1] * out_ap.ap[1][1] == num_idxs
    assert in_ap.ap[0][0] == elem_step
    stride_bytes = elem_step * mybir.dt.size(in_ap.dtype)
    stride_bytes_256 = stride_bytes // 256
    assert stride_bytes_256 * 256 == stride_bytes and stride_bytes_256 < 256
    _in_ap = gp.lower_ap_dma(in_ap, for_custom_bir_dma=True)
    return gp.add_instruction(
        mybir.InstDMAGatherAnt(
            name=gp.bass.get_next_instruction_name(),
            ins=[*_in_ap, gp.lower_ap(idxs_ap),
                 gp.lower_val_access(gp.to_reg(num_idxs))],
            outs=[gp.lower_ap(out_ap)],
            transpose=False,
            num_idxs=num_idxs,
            elem_size=elem_size,
            stride_bytes_256=stride_bytes_256,
            gen_mode=0,
            single_packet=True,
            queue_num=queue_num,
            sbuf_tokens_per_rank=0,
            sbuf_free_dim_per_rank=0,
            sbuf_free_dim_pad_per_rank=0,
            sbuf_byte_offset=0,
        )
    )


def _wrap16(vals_pb):
    """[128, btot] logical (partition, block) values -> [16, btot*8] int16
    dma_gather index layout (16-wrapped; replicate to 8 channel groups on
    device)."""
    btot = vals_pb.shape[1]
    flat = vals_pb.T.reshape(-1)  # logical position q = blk*128 + p
    return flat.reshape(btot * 8, 16).T.astype(np.int16)  # [16, btot*8]


# ---------------- host-side routing ----------------
def _route_edges(edge_index: np.ndarray):
    """Route edges (plus self-loops) to the core owning their dst; order each
    dst-tile's edges by src-table half so gather calls are half-homogeneous.

    Returns per-core index arrays plus the shared block/run schedule.
    """
    src = np.concatenate([edge_index[0].astype(np.int64), np.arange(N, dtype=np.int64)])
    dst = np.concatenate([edge_index[1].astype(np.int64), np.arange(N, dtype=np.int64)])
    core = dst // NSH
    dloc = dst - core * NSH
    g_row = (src // NSH) * NSH_PAD + (src % NSH)  # row in AllGathered table
    half = (g_row >= HALF).astype(np.int64)

    # per (core, tile, half) edge lists
    buckets = {}
    counts = np.zeros((NCORES, NT, 2), dtype=np.int64)
    for c in range(NCORES):
        m = core == c
        gr_c, dl_c, hf_c = g_row[m], dloc[m], half[m]
        key = dl_c // P * 2 + hf_c
        order = np.argsort(key, kind="stable")
        gr_c, dl_c, hf_c = gr_c[order], dl_c[order], hf_c[order]
        key = key[order]
        bounds = np.searchsorted(key, np.arange(NT * 2 + 1))
        for t in range(NT):
            for hf in range(2):
                lo, hi = bounds[t * 2 + hf], bounds[t * 2 + hf + 1]
                buckets[(c, t, hf)] = (gr_c[lo:hi], dl_c[lo:hi])
                counts[c, t, hf] = hi - lo

    B = -(-counts.max(axis=0) // P)  # [NT, 2] blocks per (tile, half)
    # block schedule: per tile, halves ordered to merge runs across tiles
    sched = []  # (tile, half)
    for t in range(NT):
        order = (0, 1) if t % 2 == 0 else (1, 0)
        for hf in order:
            sched.extend([(t, hf)] * int(B[t, hf]))
    # pad to a multiple of G, extending the final (tile, half) span
    btot = len(sched)
    pad = (-btot) % G
    sched.extend([sched[-1]] * pad)
    btot = len(sched)

    tile_of_block = np.array([t for t, _ in sched])
    half_of_block = np.array([hf for _, hf in sched])
    first = np.zeros(btot, dtype=bool)
    last = np.zeros(btot, dtype=bool)
    for t in range(NT):
        w = np.where(tile_of_block == t)[0]
        first[w[0]] = True
        last[w[-1]] = True

    # gather runs: maximal same-half spans, split at group boundaries
    runs = []  # (start_block, n_blocks, half)
    b = 0
    while b < btot:
        e = b + 1
        while (e < btot and half_of_block[e] == half_of_block[b]
               and e % G != 0):
            e += 1
        runs.append((b, e - b, int(half_of_block[b])))
        b = e

    # per-core per-(p, block) values
    srcv = np.zeros((NCORES, 128, btot), dtype=np.int64)
    dstl = np.full((NCORES, 128, btot), -1.0, dtype=np.float32)
    adst = np.zeros((NCORES, 128, btot), dtype=np.int64)
    blk_start = {}
    pos = 0
    for i, (t, hf) in enumerate(sched):
        if (t, hf) not in blk_start:
            blk_start[(t, hf)] = i
    for c in range(NCORES):
        for t in range(NT):
            for hf in range(2):
                gr, dl = buckets[(c, t, hf)]
                n_e = len(gr)
                if n_e == 0:
                    continue
                b0 = blk_start[(t, hf)]
                j = b0 * 128 + np.arange(n_e)
                srcv[c, j % 128, j // 128] = gr - hf * HALF
                dstl[c, j % 128, j // 128] = (dl - t * P).astype(np.float32)
                adst[c, j % 128, j // 128] = dl
    return srcv, dstl, adst, tile_of_block, first, last, runs, btot


# ---------------- device program ----------------
def _build_program(tile_of_block, first, last, runs, btot, phases=5):
    import contextlib

    nc = bacc.Bacc(
        "TRN2",
        target_bir_lowering=False,
        debug=False,
        enable_asserts=False,
        num_devices=NCORES,
        num_swdge_queues=4,
    )
    ngroups = btot // G
    qrr = [0]  # round-robin SWDGE queue assignment for gather calls

    def next_q():
        qrr[0] = (qrr[0] + 1) % 4
        return qrr[0]

    # I/O (xT, weights, and tables in bf16; biases f32)
    xT = nc.dram_tensor("xT", [D_IN, NSH_PAD], bf16, kind="ExternalInput")
    w1 = nc.dram_tensor("w1", [D_IN, WCOL], bf16, kind="ExternalInput")
    w2 = nc.dram_tensor("w2", [D, WCOL], bf16, kind="ExternalInput")
    b1d = nc.dram_tensor("b1", [P, D], f32, kind="ExternalInput")
    b2d = nc.dram_tensor("b2", [P, HID], f32, kind="ExternalInput")
    iotad = nc.dram_tensor("iota", [P, P], bf16, kind="ExternalInput")
    srcd = nc.dram_tensor("srci", [16, btot * 8], i16, kind="ExternalInput")
    dstld = nc.dram_tensor("dstl", [128, btot], bf16, kind="ExternalInput")
    adstd = nc.dram_tensor("adst", [16, btot * 8], i16, kind="ExternalInput")
    outd = nc.dram_tensor("out", [NSH_PAD, HID], f32, kind="ExternalOutput")

    # internal DRAM
    haug_sh = [
        nc.dram_tensor(f"haug{l}_sh", [NSH_PAD, ROWP], bf16, kind="Internal")
        for l in (1, 2)
    ]
    aldst_sh = [
        nc.dram_tensor(f"aldst{l}_sh", [NSH_PAD, ADW], bf16, kind="Internal")
        for l in (1, 2)
    ]
    haug_full = [
        nc.dram_tensor(
            f"haug{l}_full", [NFULL, ROWP], bf16, kind="Internal",
            addr_space="Shared",
        )
        for l in (1, 2)
    ]

    rg = [list(range(NCORES))]

    with tile.TileContext(nc) as tc, contextlib.ExitStack() as ctx:
        const = ctx.enter_context(tc.tile_pool(name="const", bufs=1))
        sb = ctx.enter_context(tc.tile_pool(name="sb", bufs=2))
        sb3 = ctx.enter_context(tc.tile_pool(name="sb3", bufs=3))
        ps = ctx.enter_context(tc.tile_pool(name="ps", bufs=2, space="PSUM"))

        # constants
        ident = const.tile([P, P], f32)
        make_identity(nc, ident[:])
        w1t = const.tile([P, 2, WCOL], bf16)
        nc.sync.dma_start(w1t[:], w1.ap().rearrange("(k p) n -> p k n", p=P))
        w2t = const.tile([P, 2, WCOL], bf16)
        nc.sync.dma_start(w2t[:], w2.ap().rearrange("(k p) n -> p k n", p=P))
        b1t = const.tile([P, D], f32)
        nc.sync.dma_start(b1t[:], b1d.ap())
        b2t = const.tile([P, HID], f32)
        nc.sync.dma_start(b2t[:], b2d.ap())
        iota = const.tile([P, P], bf16)
        nc.sync.dma_start(iota[:], iotad.ap())
        srci_sb = const.tile([128, btot * 8], i16)
        adst_sb = const.tile([128, btot * 8], i16)
        for r in range(8):
            nc.sync.dma_start(srci_sb[r * 16:(r + 1) * 16, :], srcd.ap())
            nc.sync.dma_start(adst_sb[r * 16:(r + 1) * 16, :], adstd.ap())
        dstl_sb = const.tile([128, btot], bf16)
        nc.sync.dma_start(dstl_sb[:], dstld.ap())

        def dense_tile(nt, lhsT_k0, lhsT_k1, wt, layer):
            """[128 nodes] x Waug matmul -> write haug/aldst shard rows."""
            d_ps = ps.tile([P, WCOL], f32, tag="dmm", name="d_ps")
            nc.tensor.matmul(d_ps[:], lhsT=lhsT_k0, rhs=wt[:, 0, :], start=True,
                             stop=False)
            nc.tensor.matmul(d_ps[:], lhsT=lhsT_k1, rhs=wt[:, 1, :], start=False,
                             stop=True)
            hb = sb.tile([P, WCOL], bf16, tag="hb", name="hb")
            nc.vector.tensor_copy(hb[:], d_ps[:])
            r0, r1 = nt * P, (nt + 1) * P
            nc.sync.dma_start(haug_sh[layer].ap()[r0:r1, 0:ROW], hb[:, 0:ROW])
            nc.sync.dma_start(aldst_sh[layer].ap()[r0:r1, 0:HEADS],
                              hb[:, ROW:WCOL])

        # ---- dense layer 1: own shard only ([h | al_src | al_dst])
        with nc.named_scope("dense1"):
            xTr = xT.ap().rearrange("(k p) n -> p k n", p=P)
            for nt in range(NT):
                xt = sb.tile([P, 2, P], bf16, tag="xt", name="xt")
                nc.sync.dma_start(xt[:], xTr[:, :, nt * P:(nt + 1) * P])
                dense_tile(nt, xt[:, 0, :], xt[:, 1, :], w1t, 0)

        def all_gather(layer):
            nc.gpsimd.collective_compute(
                "AllGather",
                mybir.AluOpType.bypass,
                replica_groups=rg,
                ins=[haug_sh[layer].ap()],
                outs=[haug_full[layer].ap()],
            )

        # SWDGE descriptor ring holds 128 descs/engine; cap calls well below.
        CHUNK = int(os.environ.get("CHUNK", "8"))  # blocks per dma_gather call

        def edge_phase(layer, flush):
            acc = {}
            run_i = 0
            for q in range(ngroups):
                qs = q * G
                hs = sb.tile([128, G, ROW], bf16, tag="hs", name="hs")
                while run_i < len(runs) and runs[run_i][0] < qs + G:
                    b0, nb, hf = runs[run_i]
                    for c0 in range(0, nb, CHUNK):
                        cb0, cnb = b0 + c0, min(CHUNK, nb - c0)
                        _ant_dma_gather(
                            nc.gpsimd,
                            out_ap=hs[:, cb0 - qs:cb0 - qs + cnb, :],
                            in_ap=haug_full[layer].ap()[
                                hf * HALF:(hf + 1) * HALF, :],
                            idxs_ap=srci_sb[:, cb0 * 8:(cb0 + cnb) * 8],
                            num_idxs=cnb * 128,
                            elem_size=ROW,
                            elem_step=ROWP,
                            queue_num=next_q(),
                        )
                    run_i += 1
                ad = sb.tile([128, G, 8], bf16, tag="ad", name="ad")
                for c0 in range(0, G, CHUNK):
                    cnb = min(CHUNK, G - c0)
                    _ant_dma_gather(
                        nc.gpsimd,
                        out_ap=ad[:, c0:c0 + cnb, :],
                        in_ap=aldst_sh[layer].ap(),
                        idxs_ap=adst_sb[:, (qs + c0) * 8:(qs + c0 + cnb) * 8],
                        num_idxs=cnb * 128,
                        elem_size=8,
                        elem_step=ADW,
                        queue_num=next_q(),
                    )
                # logits = leaky_relu(al_src[src] + al_dst[dst]); w = exp(logits)
                lg = sb.tile([128, G, HEADS], f32, tag="lg", name="lg")
                nc.vector.tensor_tensor(
                    out=lg[:], in0=hs[:, :, D:ROW], in1=ad[:],
                    op=mybir.AluOpType.add,
                )
                lr = sb.tile([128, G, HEADS], f32, tag="lr", name="lr")
                nc.vector.tensor_scalar_mul(lr[:], lg[:], NEG_SLOPE)
                nc.vector.tensor_tensor(
                    out=lr[:], in0=lg[:], in1=lr[:], op=mybir.AluOpType.max
                )
                w = sb.tile([128, G, HEADS], bf16, tag="w", name="w")
                nc.scalar.activation(w[:], lr[:],
                                     mybir.ActivationFunctionType.Exp)
                # scale h rows by the per-head weight; stash w in the al_src slot
                nc.vector.tensor_tensor(
                    out=hs[:, :, 0:D].rearrange("p g (h c) -> p g h c", c=HID),
                    in0=hs[:, :, 0:D].rearrange("p g (h c) -> p g h c", c=HID),
                    in1=w[:].unsqueeze(3).to_broadcast([128, G, HEADS, HID]),
                    op=mybir.AluOpType.mult,
                )
                nc.vector.tensor_copy(hs[:, :, D:ROW], w[:])
                # selection matrix S[p, :, d] = (dstl[p, :] == d)
                s = sb.tile([128, G, P], bf16, tag="s", name="s")
                nc.vector.tensor_tensor(
                    out=s[:],
                    in0=dstl_sb[:, qs:qs + G].unsqueeze(2).to_broadcast([128, G, P]),
                    in1=iota[:].unsqueeze(1).to_broadcast([128, G, P]),
                    op=mybir.AluOpType.is_equal,
                )
                for g in range(G):
                    b = qs + g
                    t = int(tile_of_block[b])
                    if first[b]:
                        acc[t] = ps.tile([P, ROW], f32, tag="acc", name=f"acc{t}")
                    nc.tensor.matmul(
                        acc[t][:], lhsT=s[:, g, :], rhs=hs[:, g, :],
                        start=bool(first[b]), stop=bool(last[b]),
                    )
                    if last[b]:
                        flush(t, acc.pop(t))

        # ---- layer-1 flush: normalize + relu, then fused dense layer 2 ----
        def flush1(t, acc_ps):
            rz = sb.tile([P, HEADS], f32, tag="rz", name="rz")
            nc.vector.reciprocal(rz[:], acc_ps[:, D:ROW])
            h1 = sb3.tile([P, D], f32, tag="h1", name="h1")
            nc.vector.tensor_tensor(
                out=h1[:].rearrange("p (h c) -> p h c", c=HID),
                in0=acc_ps[:, 0:D].rearrange("p (h c) -> p h c", c=HID),
                in1=rz[:].unsqueeze(2).to_broadcast([P, HEADS, HID]),
                op=mybir.AluOpType.mult,
            )
            nc.vector.tensor_tensor(
                out=h1[:], in0=h1[:], in1=b1t[:], op=mybir.AluOpType.add
            )
            nc.vector.tensor_scalar_max(h1[:], h1[:], 0.0)
            # transpose h1 -> lhsT tiles for the layer-2 dense matmul
            tp0 = ps.tile([P, P], f32, tag="tp", name="tp0")
            nc.tensor.transpose(tp0[:], h1[:, 0:P], ident[:])
            l0 = sb.tile([P, P], bf16, tag="l0", name="l0")
            nc.vector.tensor_copy(l0[:], tp0[:])
            tp1 = ps.tile([P, P], f32, tag="tp", name="tp1")
            nc.tensor.transpose(tp1[:], h1[:, P:D], ident[:])
            l1 = sb.tile([P, P], bf16, tag="l1", name="l1")
            nc.vector.tensor_copy(l1[:], tp1[:])
            dense_tile(t, l0[:], l1[:], w2t, 1)

        # ---- layer-2 flush: normalize, mean over heads, + b2, store ----
        def flush2(t, acc_ps):
            rz = sb.tile([P, HEADS], f32, tag="rz", name="rz")
            nc.vector.reciprocal(rz[:], acc_ps[:, D:ROW])
            nc.vector.tensor_scalar_mul(rz[:], rz[:], 1.0 / HEADS)
            t2 = sb.tile([P, D], f32, tag="t2", name="t2")
            nc.vector.tensor_tensor(
                out=t2[:].rearrange("p (h c) -> p h c", c=HID),
                in0=acc_ps[:, 0:D].rearrange("p (h c) -> p h c", c=HID),
                in1=rz[:].unsqueeze(2).to_broadcast([P, HEADS, HID]),
                op=mybir.AluOpType.mult,
            )
            o = sb.tile([P, HID], f32, tag="o", name="o")
            nc.vector.tensor_reduce(
                out=o[:],
                in_=t2[:].rearrange("p (h c) -> p c h", c=HID),
                axis=mybir.AxisListType.X,
                op=mybir.AluOpType.add,
            )
            nc.vector.tensor_tensor(
                out=o[:], in0=o[:], in1=b2t[:], op=mybir.AluOpType.add
            )
            nc.sync.dma_start(outd.ap()[t * P:(t + 1) * P, :], o[:])

        if phases >= 2:
            with nc.named_scope("ag1"):
                all_gather(0)
        if phases >= 3:
            with nc.named_scope("edge1"):
                edge_phase(0, flush1)
        if phases >= 4:
            with nc.named_scope("ag2"):
                all_gather(1)
        if phases >= 5:
            with nc.named_scope("edge2"):
                edge_phase(1, flush2)

    nc.compile()
    return nc


# ---------------- public entry point ----------------
_CACHE = {}


def _prepare(edge_index):
    key = edge_index.tobytes()[:1024], int(edge_index.sum())
    if _CACHE.get("key") == key:
        return _CACHE["val"]
    srcv, dstl, adst, tile_of_block, first, last, runs, btot = _route_edges(
        np.asarray(edge_index)
    )
    nc = _build_program(tile_of_block, first, last, runs, btot)
    _CACHE["key"] = key
    _CACHE["val"] = (srcv, dstl, adst, btot, nc)
    return _CACHE["val"]


def _waug(W, a_src, a_dst):
    W = np.asarray(W, np.float32)
    asrc_m = np.zeros((D, HEADS), np.float32)
    adst_m = np.zeros((D, HEADS), np.float32)
    for h in range(HEADS):
        asrc_m[h * HID:(h + 1) * HID, h] = np.asarray(a_src, np.float32)[h]
        adst_m[h * HID:(h + 1) * HID, h] = np.asarray(a_dst, np.float32)[h]
    return np.concatenate([W, W @ asrc_m, W @ adst_m], axis=1)


def _make_in_maps(inputs, srcv, dstl, adst):
    x = np.asarray(inputs["x"], np.float32)
    w1_np = _waug(inputs["W1"], inputs["a_src1"], inputs["a_dst1"]).astype(bfnp)
    w2_np = _waug(inputs["W2"], inputs["a_src2"], inputs["a_dst2"]).astype(bfnp)
    b1_np = np.tile(np.asarray(inputs["b1"], np.float32).reshape(1, D), (P, 1))
    b2_np = np.tile(np.asarray(inputs["b2"], np.float32).reshape(1, HID), (P, 1))
    iota_np = np.tile(np.arange(P, dtype=np.float32).reshape(1, P),
                      (P, 1)).astype(bfnp)

    xt_full = np.ascontiguousarray(x.T)  # [256, 50000]
    xT = np.zeros((NCORES, D_IN, NSH_PAD), bfnp)
    for c in range(NCORES):
        xT[c, :, :NSH] = xt_full[:, c * NSH:(c + 1) * NSH].astype(bfnp)

    return [
        {
            "xT": np.ascontiguousarray(xT[c]),
            "w1": w1_np,
            "w2": w2_np,
            "b1": b1_np,
            "b2": b2_np,
            "iota": iota_np,
            "srci": _wrap16(srcv[c]),
            "dstl": np.ascontiguousarray(dstl[c].astype(bfnp)),
            "adst": _wrap16(adst[c]),
        }
        for c in range(NCORES)
    ]


def kernel(
    x, edge_index, W1, a_src1, a_dst1, b1, W2, a_src2, a_dst2, b2
) -> np.ndarray:
    inputs = dict(x=x, W1=W1, a_src1=a_src1, a_dst1=a_dst1, b1=b1,
                  W2=W2, a_src2=a_src2, a_dst2=a_dst2, b2=b2)
    srcv, dstl, adst, btot, nc = _prepare(np.asarray(edge_index))
    in_maps = _make_in_maps(inputs, srcv, dstl, adst)
    res = bass_utils.run_bass_kernel_spmd(nc, in_maps, core_ids=list(range(NCORES)))
    out = np.concatenate(
        [np.asarray(res.results[c]["out"])[:NSH] for c in range(NCORES)], axis=0
    )
    return out
